# revision 14
# baseline (speedup 1.0000x reference)
"""Grouped 3x3 SAME conv on 8 Trainium2 NeuronCores.

Problem: x[16,56,56,256] NHWC, 8 groups of 32->64 channels, 3x3 SAME,
out[16,56,56,512], fp32.

Strategy (hardcoded):
  - Data-parallel over batch: core i handles images [2i, 2i+1].
  - Host-side layout prep (part of the sharding step): transpose x to
    channels-major, zero-pad spatial to 58x58, pre-replicate the three
    kh-shifted copies, and cast to fp16 (11-bit mantissa; conv accumulates
    in fp32 PSUM, so rel err stays ~5e-4). Device output comes back
    channels-major fp32 and the host transposes back to NHWC.
  - On device: conv = matmuls with contraction stacked over (kh, c) = 96
    partitions; the kw shift is a +-1 column offset on the same SBUF tile.
    Two groups are packed per wave via tile_position col-groups (0,0) and
    (0,64) writing one PSUM [128, N] tile; fp16 streams 1 cycle/row and
    allows N up to 1024, so spatial tiles are 16 image rows (N=928).
    Bias is added by DVE during the PSUM->SBUF copy.
"""

import numpy as np

G = 8        # groups
P = 32       # in-channels per group
F = 64       # out-channels per group
H = W = 56
HP = WP = 58           # zero-padded spatial
SP = HP * WP           # 3364 padded pixels
SHIFT = WP             # column shift of one image row
N_CORES = 8
B_PER_CORE = 2
NPAIR = G // 2         # group pairs packed per wave
# spatial tiles over padded cols [58, 3306): 8 image rows each
# (N=464 <= 512: a matmul writes one PSUM bank)
TILES = [((1 + 8 * t) * SHIFT, 8 * SHIFT) for t in range(7)]

_PROG_CACHE = {}


def _build_program():
    import concourse.bacc as bacc
    import concourse.mybir as mybir
    import concourse.tile as tile

    dt = mybir.dt
    nc = bacc.Bacc(
        "TRN2",
        target_bir_lowering=False,
        debug=False,
        num_devices=N_CORES,
    )

    f32 = dt.float32
    f16 = dt.float16

    xT = nc.dram_tensor("xT", [B_PER_CORE, G, 3 * P, SP], f16,
                        kind="ExternalInput")
    wT = nc.dram_tensor("wT", [3 * P, G * 3 * F], f16,
                        kind="ExternalInput")
    bT = nc.dram_tensor("bT", [2 * F, NPAIR], f32, kind="ExternalInput")
    outT = nc.dram_tensor("outT", [B_PER_CORE, G * F, SP], f32,
                          kind="ExternalOutput")

    with tile.TileContext(nc) as tc:
        with (
            tc.tile_pool(name="const", bufs=1) as cpool,
            tc.tile_pool(name="xg", bufs=10) as xpool,
            tc.tile_pool(name="ot", bufs=6) as opool,
            tc.tile_pool(name="ps", bufs=4, space="PSUM") as ppool,
        ):
            wsb = cpool.tile([3 * P, G * 3 * F], f16)
            nc.scalar.dma_start(wsb[:], wT[:])
            bsb = cpool.tile([2 * F, NPAIR], f32)
            nc.scalar.dma_start(bsb[:], bT[:])

            for b in range(B_PER_CORE):
                for gp in range(NPAIR):
                    ga, gb = 2 * gp, 2 * gp + 1
                    for s, nt in TILES:
                        # per-tile activation chunks: [96, nt+2] = 3
                        # kh-shifted replicas (host pre-replicated), with
                        # one extra col on each side for the kw shifts
                        xa = xpool.tile([3 * P, 8 * SHIFT + 2], f16,
                                        tag="xa")
                        xb = xpool.tile([3 * P, 8 * SHIFT + 2], f16,
                                        tag="xb")
                        nc.sync.dma_start(xa[:, :nt + 2],
                                          xT[b, ga, :, s - 1:s + 1 + nt])
                        nc.sync.dma_start(xb[:, :nt + 2],
                                          xT[b, gb, :, s - 1:s + 1 + nt])
                        ps = ppool.tile([2 * F, 8 * SHIFT], f32)
                        for dw in range(3):
                            nc.tensor.matmul(
                                ps[0:F, :nt],
                                wsb[:, (ga * 3 + dw) * F:(ga * 3 + dw + 1) * F],
                                xa[:, dw:dw + nt],
                                start=(dw == 0),
                                stop=(dw == 2),
                                tile_position=(0, 0),
                            )
                            nc.tensor.matmul(
                                ps[F:2 * F, :nt],
                                wsb[:, (gb * 3 + dw) * F:(gb * 3 + dw + 1) * F],
                                xb[:, dw:dw + nt],
                                start=(dw == 0),
                                stop=(dw == 2),
                                tile_position=(0, F),
                            )
                        ot = opool.tile([2 * F, 8 * SHIFT], f32)
                        nc.vector.tensor_scalar_add(ot[:, :nt], ps[:, :nt],
                                                    bsb[:, gp:gp + 1])
                        nc.scalar.dma_start(
                            outT[b, gp * 2 * F:(gp + 1) * 2 * F, s:s + nt],
                            ot[:, :nt])

    nc.compile()
    return nc


def _get_program():
    if "nc" not in _PROG_CACHE:
        _PROG_CACHE["nc"] = _build_program()
    return _PROG_CACHE["nc"]


def prepare_in_maps(x, kernels, bias):
    x = np.ascontiguousarray(x, dtype=np.float32)
    kernels = np.ascontiguousarray(kernels, dtype=np.float32)
    bias = np.ascontiguousarray(bias, dtype=np.float32)

    nb = x.shape[0]
    # zero-padded channels-major view of x: [b, g, c, hp*wp], fp16
    xpad = np.zeros((nb, G, P, HP, WP), np.float16)
    xpad[:, :, :, 1:1 + H, 1:1 + W] = (
        x.transpose(0, 3, 1, 2).reshape(nb, G, P, H, W).astype(np.float16)
    )
    xpad = xpad.reshape(nb, G, P, SP)
    # pre-replicated kh-shifted blocks: xT[b,g,32j+c,m] = xpad[...,m+58(j-1)]
    xT = np.zeros((nb, G, 3, P, SP), np.float16)
    xT[:, :, 0, :, SHIFT:] = xpad[:, :, :, :SP - SHIFT]
    xT[:, :, 1, :, :] = xpad
    xT[:, :, 2, :, :SP - SHIFT] = xpad[:, :, :, SHIFT:]
    xT = xT.reshape(nb, G, 3 * P, SP)
    # [kh*c, g*kw*f] weight layout: lhsT slices [96, 64] per (g, kw)
    wT = np.ascontiguousarray(
        kernels.transpose(1, 3, 0, 2, 4).reshape(3 * P, G * 3 * F)
    ).astype(np.float16)
    bT = np.ascontiguousarray(bias.reshape(NPAIR, 2 * F).T)

    return [
        {"xT": np.ascontiguousarray(xT[i * B_PER_CORE:(i + 1) * B_PER_CORE]),
         "wT": wT, "bT": bT}
        for i in range(N_CORES)
    ]


def gather_output(results, nb):
    out = np.empty((nb, H, W, G * F), np.float32)
    for i in range(N_CORES):
        o = results[i]["outT"].reshape(B_PER_CORE, G * F, HP, WP)
        o = o[:, :, 1:1 + H, 1:1 + W]               # drop padded rows/cols
        out[i * B_PER_CORE:(i + 1) * B_PER_CORE] = o.transpose(0, 2, 3, 1)
    return out


def kernel(x, kernels, bias):
    from concourse.bass_utils import run_bass_kernel_spmd

    nc = _get_program()
    in_maps = prepare_in_maps(x, kernels, bias)
    res = run_bass_kernel_spmd(nc, in_maps, list(range(N_CORES)))
    return gather_output(res.results, np.asarray(x).shape[0])
